# revision 12
# baseline (speedup 1.0000x reference)
"""Euler characteristic curve (cubical complex) kernel for Trainium2.

Problem: x [32,16,128,128] f32 -> ECC [32,16,64] f32.
Per (b,c) slice: every cell of the 255x255 vertex-mode cubical grid has
filtration bin K = ceil(63*max(corner values)) in [0,63];
ECC(t) = #V(K<=t) - #Eh(K<=t) - #Ev(K<=t) + #Q(K<=t).

Strategy (per core, 64 slices, pure data parallel over 8 cores):
 - Lower-star compression: assign each cell to the lexicographically-first
   corner achieving its max bin. Every cell assigned to vertex v activates at
   bin K_v, so chi(t) = sum_v w_v * [K_v <= t] with a t-independent integer
   weight w_v = 1 - (#edges assigned) + (#squares assigned) in [-3, 1].
   This turns 4 cell arrays into ONE weighted vertex array for the DVE.
 - Layout: partition p = (slice s=p//2, half h=p%2); each partition holds 64
   image rows + up/down overlap rows, row stride 130 (128 cols + 2 sentinel
   cols, sentinel value 1024 in bin domain > 63 self-excludes from counts and
   kills cross-boundary cells in the beat comparisons).
 - Exact binning: y=63*x on ACT (exact fp32 FMA), int-cast ceil trick on DVE.
 - Per DVE threshold t: mask = (K is_le t) @4x, mask*w @2x, PE column-sum
   matmuls (blocksel weights, 17 chunks) accumulate per-slice sums in PSUM,
   ACT Copy+accum_out tail-reduces PSUM -> chi[:, t]. Exact int arithmetic.
 - Remaining thresholds on ACT: Sign(X - t - .5) + accum over the 4 plain
   cell arrays (V=K, Eh, Ev, Q maxes); host decodes counts from sign-sums.
 - t = 63: chi = Euler characteristic of the full square = 1 (host constant).
"""

import numpy as np

B, C, H, W = 32, 16, 128, 128
RES = 64
NCORES = 8
SLICES = B * C              # 512
SPC = SLICES // NCORES      # 64 slices per core
NPART = 128

SW = 130                    # row stride: 128 cols + 2 sentinel columns
ROWS = 67                   # pad row + up-overlap + 64 owned + down-overlap
WTOT = ROWS * SW            # 8710 input width per partition
KW = WTOT + 4               # K tile width (pad, memset to sentinel)
OWN = 260                   # owned rows start (flat offset, row 2)
OWN_W = 64 * SW             # 8320 owned width
EHX_W = 8582                # Ehx width (Eh over flat 129..8710, padded even)
XSENT = 20.25               # x-domain sentinel -> K = ceil(63*20.25) = 1276
NMM = 17                    # 16x512 + 1x128 moving chunks per threshold


def _dve_sourced(t: int) -> bool:
    """Mask source for threshold t: True -> DVE ts(is_le); False -> ACT Sign."""
    return t % 6 == 3


_CACHE = {}


def _build_program(legalize=True):
    import concourse.bass as bass
    import concourse.mybir as mybir
    from concourse.tile import TileContext
    from concourse.alu_op_type import AluOpType as alu

    dt = mybir.dt
    af = mybir.ActivationFunctionType
    nc = bass.Bass("TRN2", target_bir_lowering=False, debug=False)

    x_dram = nc.dram_tensor("xi", [NPART, WTOT], dt.float32, kind="ExternalInput").ap()
    bsel_dram = nc.dram_tensor("bsel", [NPART, 64], dt.float32, kind="ExternalInput").ap()
    bias_dram = nc.dram_tensor("bias", [NPART, 64], dt.float32, kind="ExternalInput").ap()
    chi_dram = nc.dram_tensor("chi", [SPC, 64], dt.float32, kind="ExternalOutput").ap()

    HCH = 4356  # K-compute column chunk width (even; chunks 4356 + 4354)

    with TileContext(nc) as tc:
        with (
            tc.tile_pool(name="persist", bufs=1) as ap_,
            tc.tile_pool(name="ps", bufs=4, space="PSUM") as pp,
        ):
            # ---- persistent tiles ----
            K = ap_.tile([NPART, KW], dt.bfloat16)
            w = ap_.tile([NPART, OWN_W], dt.bfloat16)
            ehx = ap_.tile([NPART, EHX_W], dt.bfloat16)
            q = ap_.tile([NPART, OWN_W], dt.bfloat16)
            bself = ap_.tile([NPART, 64], dt.float32)
            bselb = ap_.tile([NPART, 64], dt.bfloat16)
            biasT = ap_.tile([NPART, 64], dt.float32)
            chi = ap_.tile([SPC, 64], dt.float32)
            scr512 = ap_.tile([SPC, 512], dt.bfloat16)

            nc.sync.dma_start(bself[:, :], bsel_dram)
            nc.sync.dma_start(biasT[:, :], bias_dram)
            nc.vector.tensor_copy(bselb[:, :], bself[:, :])
            nc.vector.memset(K[:, WTOT:KW], 1276.0)

            # ---- K = ceil(63*x), exact (ACT mult + int-cast ceil) ----
            with tc.tile_pool(name="kprep", bufs=1) as kp:
                xf = kp.tile([NPART, WTOT], dt.float32)
                ft = kp.tile([NPART, 3 * HCH], dt.float32)
                ht = kp.tile([NPART, 2 * HCH], dt.bfloat16)
                nc.sync.dma_start(xf[:, 0:HCH], x_dram[:, 0:HCH])
                nc.sync.dma_start(xf[:, HCH:WTOT], x_dram[:, HCH:WTOT])
                for lo, hi in ((0, HCH), (HCH, WTOT)):
                    cw = hi - lo
                    y = ft[:, 0:cw]
                    yt = ft[:, HCH : HCH + cw]
                    ki = ft[:, 2 * HCH : 2 * HCH + cw].bitcast(dt.int32)
                    de = ht[:, 0:cw]
                    ytb = ht[:, HCH : HCH + cw]
                    nc.scalar.activation(y, xf[:, lo:hi], af.Copy, bias=0.0, scale=63.0)
                    nc.vector.tensor_copy(ki, y)                      # f32 -> int32
                    nc.vector.tensor_copy(yt, ki)                     # int32 -> f32
                    nc.vector.tensor_tensor(de, y, yt, alu.is_gt)     # 1x f32
                    nc.vector.tensor_copy(ytb, yt)                    # f32 -> bf16
                    nc.vector.tensor_tensor(K[:, lo:hi], de, ytb, alu.add)

            # ---- Eh array (flat positions 129 .. 129+EHX_W) and Q ----
            nc.vector.tensor_tensor(
                ehx[:, :], K[:, 129 : 129 + EHX_W], K[:, 130 : 130 + EHX_W], alu.max
            )
            nc.vector.tensor_tensor(
                q[:, :], ehx[:, 131 : 131 + OWN_W], ehx[:, 261 : 261 + OWN_W], alu.max
            )

            Ko = K[:, OWN : OWN + OWN_W]

            # ---- per-vertex weights w = 1 - E + S (lex-first tie-break) ----
            with tc.tile_pool(name="wprep", bufs=1) as wp:
                tt = wp.tile([NPART, 6 * OWN_W], dt.bfloat16)
                s = [tt[:, i * OWN_W : (i + 1) * OWN_W] for i in range(6)]
                TT = nc.vector.tensor_tensor
                TT(s[0], Ko, K[:, OWN - 1 : OWN - 1 + OWN_W], alu.is_gt)    # bL'
                TT(s[1], Ko, K[:, OWN + 1 : OWN + 1 + OWN_W], alu.is_ge)    # bR'
                TT(s[2], Ko, K[:, OWN - SW : OWN - SW + OWN_W], alu.is_gt)  # bU'
                TT(s[3], Ko, K[:, OWN + SW : OWN + SW + OWN_W], alu.is_ge)  # bD'
                TT(s[4], s[0], s[1], alu.add)                               # e1
                TT(s[5], s[2], s[3], alu.add)                               # e2
                TT(s[2], s[4], s[5], alu.add)                               # E
                TT(s[3], Ko, ehx[:, 0:OWN_W], alu.is_gt)                    # cUL
                TT(s[4], s[3], s[0], alu.mult)                              # S_ul
                TT(s[5], Ko, ehx[:, 1 : 1 + OWN_W], alu.is_gt)              # cUR
                TT(s[3], s[5], s[1], alu.mult)                              # S_ur
                TT(s[5], Ko, ehx[:, 260 : 260 + OWN_W], alu.is_ge)          # cLL
                TT(s[1], s[5], s[0], alu.mult)                              # S_ll
                TT(s[5], Ko, q[:, :], alu.is_ge)                            # S_lr
                TT(s[0], s[4], s[3], alu.add)                               # S_ul+S_ur
                TT(s[3], s[1], s[5], alu.add)                               # S_ll+S_lr
                TT(s[4], s[0], s[3], alu.add)                               # S
                nc.vector.tensor_scalar(s[5], s[2], -1.0, 1.0, alu.mult, alu.add)  # 1-E
                TT(s[1], s[4], s[5], alu.add)                               # w (raw)
                # zero w at sentinel positions so sign-sourced sums are clean
                nc.vector.tensor_scalar(s[0], Ko, 63.0, 1.0, alu.is_le, alu.mult)
                TT(w[:, :], s[1], s[0], alu.mult)                           # w

            # ---- threshold loop (all via mask*w -> PE colsum -> ACT tail) ----
            # mask source: DVE ts(is_le) for t in DVE_SRC; ACT Sign otherwise.
            # sign-sourced: sum = Sw - 2*chi(t) with Sw = sum_slice(w) = 1.
            with tc.tile_pool(name="thr", bufs=3) as mp:
                pending = []

                def _emit_tail(pt, ppsum):
                    nc.scalar.activation(
                        scr512[:, :], ppsum[:, :], af.Copy, bias=0.0, scale=1.0,
                        accum_out=chi[:, pt : pt + 1],
                    )

                for t in range(RES - 1):
                    m0 = mp.tile([NPART, OWN_W], dt.bfloat16, tag="m0")
                    m1 = mp.tile([NPART, OWN_W], dt.bfloat16, tag="m1")
                    if _dve_sourced(t):
                        nc.vector.tensor_scalar(m0[:, :], Ko, float(t), None, alu.is_le)
                    else:
                        nc.scalar.activation(
                            m0[:, :], Ko, af.Sign,
                            bias=biasT[:, t : t + 1], scale=1.0,
                        )
                    nc.vector.tensor_tensor(m1[:, :], m0[:, :], w[:, :], alu.mult)
                    psum = pp.tile([SPC, 512], dt.float32, tag="ps")
                    for c in range(NMM):
                        lo = 512 * c
                        hi = min(512 * (c + 1), OWN_W)
                        nc.tensor.matmul(
                            psum[:, 0 : hi - lo], bselb[:, :], m1[:, lo:hi],
                            start=(c == 0), stop=(c == NMM - 1),
                        )
                    # software-pipeline the PSUM tail-reduce: emitting it now
                    # would stall the ACT FIFO on the DVE->PE chain for t
                    pending.append((t, psum))
                    if len(pending) >= 3:
                        _emit_tail(*pending.pop(0))
                for pt, ppsum in pending:
                    _emit_tail(pt, ppsum)

            nc.sync.dma_start(chi_dram, chi[:, :])

    if legalize:
        _legalize_waits(nc)
    return nc


def _legalize_waits(nc, max_waits: int = 1):
    """This walrus build rejects instructions with more than one sync wait.
    Split excess waits onto preceding same-engine NoOps."""
    import concourse.mybir as mybir

    for f in nc.m.functions:
        for b in f.blocks:
            il = list(b.instructions)
            out, changed = [], False
            for inst in il:
                try:
                    si = inst.sync_info
                except AttributeError:
                    si = None
                waits = list(si.on_wait) if si else []
                if len(waits) > max_waits:
                    head, keep = waits[:-max_waits], waits[-max_waits:]
                    for k, wv in enumerate(head):
                        out.append(
                            mybir.InstNoOp(
                                name=f"{inst.name}-w{k}",
                                engine=inst.engine,
                                sync_info=mybir.SyncInfo(on_wait=[wv], on_update=[]),
                                bass_nofuse=True,
                            )
                        )
                    inst.sync_info = mybir.SyncInfo(
                        on_wait=keep, on_update=list(si.on_update)
                    )
                    changed = True
                out.append(inst)
            if changed:
                b.instructions = out


def make_host_inputs(xcore: np.ndarray):
    """xcore [SPC, H, W] f32 -> packed xi [NPART, WTOT] plus bsel/bias."""
    xi = np.full((SPC, 2, ROWS, SW), XSENT, dtype=np.float32)
    xh = xcore.reshape(SPC, 2, 64, W)
    xi[:, :, 2:66, 0:W] = xh
    xi[:, 1, 1, 0:W] = xcore[:, 63, :]   # h=1 up-overlap = image row 63
    xi[:, 0, 66, 0:W] = xcore[:, 64, :]  # h=0 down-overlap = image row 64
    return xi.reshape(NPART, WTOT)


def _host_bsel_bias():
    bsel = np.zeros((NPART, 64), dtype=np.float32)
    bsel[np.arange(NPART), np.arange(NPART) // 2] = 1.0
    bias = np.broadcast_to(
        -(np.arange(64, dtype=np.float32) + 0.5), (NPART, 64)
    ).copy()
    return bsel, bias


def _install_ntff_hook():
    import sys, types

    if "antenv.axon_hooks" in sys.modules:
        return
    mod = types.ModuleType("antenv.axon_hooks")
    state = {"hook": None}
    mod.set_axon_ntff_profile_hook = lambda h: state.update(hook=h)
    mod.get_axon_ntff_profile_hook = lambda: state["hook"]
    sys.modules["antenv.axon_hooks"] = mod
    try:
        from trn_agent_boot.trn_boot import _ntff_profile_via_ctypes

        hook = _ntff_profile_via_ctypes("/opt/axon/libaxon_pjrt.so")
        if hook is not None:
            mod.set_axon_ntff_profile_hook(hook)
    except Exception:
        pass


def _run(x: np.ndarray, trace: bool = False):
    from concourse import bass_utils

    if trace:
        _install_ntff_hook()

    x = np.ascontiguousarray(np.asarray(x), dtype=np.float32)
    assert x.shape == (B, C, H, W)

    if "nc" not in _CACHE:
        _CACHE["nc"] = _build_program()
    nc = _CACHE["nc"]

    bsel, bias = _host_bsel_bias()
    flat = x.reshape(SLICES, H, W)
    in_maps = []
    for k in range(NCORES):
        xi = make_host_inputs(flat[k * SPC : (k + 1) * SPC])
        in_maps.append({"xi": xi, "bsel": bsel, "bias": bias})
    res = bass_utils.run_bass_kernel_spmd(
        nc, in_maps, core_ids=list(range(NCORES)), trace=trace
    )

    ecc = np.empty((SLICES, RES), dtype=np.float64)
    for k in range(NCORES):
        chi = res.results[k]["chi"].astype(np.float64)   # [SPC, 64]
        sl = slice(k * SPC, (k + 1) * SPC)
        for t in range(RES - 1):
            if _dve_sourced(t):
                ecc[sl, t] = chi[:, t]
            else:
                # Sign mask: sum = Sw - 2*chi(t), Sw = sum(w) per slice = 1
                ecc[sl, t] = (1.0 - chi[:, t]) / 2.0
    ecc[:, RES - 1] = 1.0
    return ecc.reshape(B, C, RES).astype(np.float32), res


def kernel(x: np.ndarray) -> np.ndarray:
    out, _ = _run(x, trace=False)
    return out


# revision 13
# speedup vs baseline: 1.1821x; 1.1821x over previous
"""Euler characteristic curve (cubical complex) kernel for Trainium2.

Problem: x [32,16,128,128] f32 -> ECC [32,16,64] f32.
Per (b,c) slice: every cell of the 255x255 vertex-mode cubical grid has
filtration bin K = ceil(63*max(corner values)) in [0,63];
ECC(t) = #V(K<=t) - #Eh(K<=t) - #Ev(K<=t) + #Q(K<=t).

Strategy (per core, 64 slices, pure data parallel over 8 cores):
 - Lower-star compression: assign each cell to the lexicographically-first
   corner achieving its max bin; every cell assigned to vertex v activates at
   bin K_v, so chi(t) = sum_v w_v * [K_v <= t] with a t-independent integer
   weight w_v = 1 - (#edges assigned) + (#squares assigned) in [-3, 1].
   This turns 4 cell arrays into ONE weighted vertex array.
 - Layout: partition p = (slice s=p//2, half h=p%2); each partition holds 64
   image rows + up/down overlap rows, row stride 130 (128 cols + 2 sentinel
   cols; sentinel bin 1276 > 63 self-excludes from all counts).
 - Exact binning: y=63*x and the int-casts on ACT (any adjacent-integer cast
   rounding works for the ceil trick), compare+fix on DVE.
 - Steady state, per threshold: mask m0 = [K<=t] via DVE tensor_scalar @4x
   (some t) or ACT Sign (rest; engine balance), m1 = m0*w via DVE @2x, PE
   column-sum matmuls (constant blocksel weights) accumulate per-slice sums
   in PSUM, ACT Copy+accum_out tail-reduces PSUM -> chi[:, t] (sw-pipelined).
 - During the DVE-only weight-prep window, ACT runs a few thresholds in
   cell-mode instead: Sign+accum over the 4 plain cell arrays (V,Eh,Ev,Q).
 - All arithmetic is exact in integers; t = 63 is the Euler characteristic
   of the full square = 1 (host constant).
"""

import numpy as np

B, C, H, W = 32, 16, 128, 128
RES = 64
NCORES = 8
SLICES = B * C              # 512
SPC = SLICES // NCORES      # 64 slices per core
NPART = 128

SW = 130                    # row stride: 128 cols + 2 sentinel columns
ROWS = 67                   # pad row + up-overlap + 64 owned + down-overlap
WTOT = ROWS * SW            # 8710 input width per partition
KW = WTOT + 4               # K tile width (pad, memset to sentinel)
OWN = 260                   # owned rows start (flat offset, row 2)
OWN_W = 64 * SW             # 8320 owned width
EHX_W = 8582                # Ehx width (Eh over flat 129..8711)
XSENT = 20.25               # x-domain sentinel -> K = ceil(63*20.25) = 1276
NMM = 17                    # 16x512 + 1x128 moving chunks per threshold

CELL_TS = (15, 33, 53)      # thresholds counted in cell-mode during w-prep


def _mask_source(t: int) -> str:
    """'cell' (ACT 4-array count), 'ts' (DVE is_le), or 'sign' (ACT Sign)."""
    if t in CELL_TS:
        return "cell"
    return "ts" if t % 3 == 1 else "sign"


_CACHE = {}


def _build_program(legalize=True):
    import concourse.bass as bass
    import concourse.mybir as mybir
    from concourse.tile import TileContext
    from concourse.alu_op_type import AluOpType as alu

    dt = mybir.dt
    af = mybir.ActivationFunctionType
    nc = bass.Bass("TRN2", target_bir_lowering=False, debug=False)

    x_dram = nc.dram_tensor("xi", [NPART, WTOT], dt.float32, kind="ExternalInput").ap()
    bsel_dram = nc.dram_tensor("bsel", [NPART, 64], dt.float32, kind="ExternalInput").ap()
    bias_dram = nc.dram_tensor("bias", [NPART, 64], dt.float32, kind="ExternalInput").ap()
    chi_dram = nc.dram_tensor("chi", [SPC, 64], dt.float32, kind="ExternalOutput").ap()
    acts_dram = nc.dram_tensor(
        "acts", [NPART, 4 * len(CELL_TS)], dt.float32, kind="ExternalOutput"
    ).ap()

    HCH = 4356  # K-compute column chunk width (even; chunks 4356 + 4354)

    with TileContext(nc) as tc:
        with (
            tc.tile_pool(name="persist", bufs=1) as ap_,
            tc.tile_pool(name="ps", bufs=4, space="PSUM") as pp,
        ):
            # ---- persistent tiles ----
            K = ap_.tile([NPART, KW], dt.bfloat16)
            w = ap_.tile([NPART, OWN_W], dt.bfloat16)
            bself = ap_.tile([NPART, 64], dt.float32)
            bselb = ap_.tile([NPART, 64], dt.bfloat16)
            biasT = ap_.tile([NPART, 64], dt.float32)
            chi = ap_.tile([SPC, 64], dt.float32)
            acts = ap_.tile([NPART, 4 * len(CELL_TS)], dt.float32)
            scr512 = ap_.tile([SPC, 512], dt.bfloat16)

            nc.sync.dma_start(bself[:, :], bsel_dram)
            nc.sync.dma_start(biasT[:, :], bias_dram)
            nc.vector.tensor_copy(bselb[:, :], bself[:, :])
            nc.vector.memset(K[:, WTOT:KW], 1276.0)

            # ---- K = ceil(63*x): ACT does mult + casts, DVE compare + fix ----
            with tc.tile_pool(name="kprep", bufs=1) as kp:
                xf = kp.tile([NPART, WTOT], dt.float32)
                ft = kp.tile([NPART, 2 * HCH], dt.float32)
                it_ = kp.tile([NPART, HCH], dt.int32)
                ht = kp.tile([NPART, 2 * HCH], dt.bfloat16)
                nc.sync.dma_start(xf[:, 0:HCH], x_dram[:, 0:HCH])
                nc.sync.dma_start(xf[:, HCH:WTOT], x_dram[:, HCH:WTOT])
                for lo, hi in ((0, HCH), (HCH, WTOT)):
                    cw = hi - lo
                    y = ft[:, 0:cw]
                    yt = ft[:, HCH : HCH + cw]
                    ki = it_[:, 0:cw]
                    de = ht[:, 0:cw]
                    ytb = ht[:, HCH : HCH + cw]
                    nc.scalar.activation(y, xf[:, lo:hi], af.Copy, bias=0.0, scale=63.0)
                    nc.scalar.activation(ki, y, af.Copy)               # f32 -> int32
                    nc.scalar.activation(yt, ki, af.Copy)              # int32 -> f32
                    nc.scalar.activation(ytb, yt, af.Copy)             # f32 -> bf16
                    nc.vector.tensor_tensor(de, y, yt, alu.is_gt)      # 1x f32
                    nc.vector.tensor_tensor(K[:, lo:hi], de, ytb, alu.add)

            Ko = K[:, OWN : OWN + OWN_W]

            # ---- cell arrays + per-vertex weights; ACT runs cell-mode
            #      thresholds concurrently with the DVE-only weight prep ----
            with tc.tile_pool(name="wprep", bufs=1) as wp:
                ehx = wp.tile([NPART, EHX_W], dt.bfloat16)
                q = wp.tile([NPART, OWN_W], dt.bfloat16)
                ev = wp.tile([NPART, OWN_W], dt.bfloat16)
                ascr = wp.tile([NPART, OWN_W], dt.bfloat16)
                tt = wp.tile([NPART, 6 * OWN_W], dt.bfloat16)
                s = [tt[:, i * OWN_W : (i + 1) * OWN_W] for i in range(6)]
                TT = nc.vector.tensor_tensor

                TT(ehx[:, :], K[:, 129 : 129 + EHX_W], K[:, 130 : 130 + EHX_W], alu.max)
                TT(q[:, :], ehx[:, 131 : 131 + OWN_W], ehx[:, 261 : 261 + OWN_W], alu.max)
                TT(ev[:, :], Ko, K[:, OWN + SW : OWN + SW + OWN_W], alu.max)

                # cell-mode ACT thresholds (independent of w)
                cell_arrs = (Ko, ehx[:, 131 : 131 + OWN_W], ev[:, :], q[:, :])
                for ic, tc_ in enumerate(CELL_TS):
                    for j, arr in enumerate(cell_arrs):
                        nc.scalar.activation(
                            ascr[:, :], arr, af.Sign,
                            bias=biasT[:, tc_ : tc_ + 1], scale=1.0,
                            accum_out=acts[:, 4 * ic + j : 4 * ic + j + 1],
                        )

                TT(s[0], Ko, K[:, OWN - 1 : OWN - 1 + OWN_W], alu.is_gt)    # bL'
                TT(s[1], Ko, K[:, OWN + 1 : OWN + 1 + OWN_W], alu.is_ge)    # bR'
                TT(s[2], Ko, K[:, OWN - SW : OWN - SW + OWN_W], alu.is_gt)  # bU'
                TT(s[3], Ko, K[:, OWN + SW : OWN + SW + OWN_W], alu.is_ge)  # bD'
                TT(s[4], s[0], s[1], alu.add)                               # e1
                TT(s[5], s[2], s[3], alu.add)                               # e2
                TT(s[2], s[4], s[5], alu.add)                               # E
                TT(s[3], Ko, ehx[:, 0:OWN_W], alu.is_gt)                    # cUL
                TT(s[4], s[3], s[0], alu.mult)                              # S_ul
                TT(s[5], Ko, ehx[:, 1 : 1 + OWN_W], alu.is_gt)              # cUR
                TT(s[3], s[5], s[1], alu.mult)                              # S_ur
                TT(s[5], Ko, ehx[:, 260 : 260 + OWN_W], alu.is_ge)          # cLL
                TT(s[1], s[5], s[0], alu.mult)                              # S_ll
                TT(s[5], Ko, q[:, :], alu.is_ge)                            # S_lr
                TT(s[0], s[4], s[3], alu.add)                               # S_ul+S_ur
                TT(s[3], s[1], s[5], alu.add)                               # S_ll+S_lr
                TT(s[4], s[0], s[3], alu.add)                               # S
                nc.vector.tensor_scalar(s[5], s[2], -1.0, 1.0, alu.mult, alu.add)  # 1-E
                TT(s[1], s[4], s[5], alu.add)                               # w (raw)
                # zero w at sentinel positions so sign-sourced sums are clean
                nc.vector.tensor_scalar(s[0], Ko, 63.0, 1.0, alu.is_le, alu.mult)
                TT(w[:, :], s[1], s[0], alu.mult)                           # w

            # ---- threshold loop (mask*w -> PE colsum -> ACT tail) ----
            # sign-sourced decode: sum = Sw - 2*chi(t) with Sw = sum(w) = 1
            with tc.tile_pool(name="thr", bufs=3) as mp:
                pending = []

                def _emit_tail(pt, ppsum):
                    nc.scalar.activation(
                        scr512[:, :], ppsum[:, :], af.Copy, bias=0.0, scale=1.0,
                        accum_out=chi[:, pt : pt + 1],
                    )

                for t in range(RES - 1):
                    src = _mask_source(t)
                    if src == "cell":
                        continue
                    m0 = mp.tile([NPART, OWN_W], dt.bfloat16, tag="m0")
                    m1 = mp.tile([NPART, OWN_W], dt.bfloat16, tag="m1")
                    if src == "ts":
                        nc.vector.tensor_scalar(m0[:, :], Ko, float(t), None, alu.is_le)
                    else:
                        nc.scalar.activation(
                            m0[:, :], Ko, af.Sign,
                            bias=biasT[:, t : t + 1], scale=1.0,
                        )
                    nc.vector.tensor_tensor(m1[:, :], m0[:, :], w[:, :], alu.mult)
                    psum = pp.tile([SPC, 512], dt.float32, tag="ps")
                    for c in range(NMM):
                        lo = 512 * c
                        hi = min(512 * (c + 1), OWN_W)
                        nc.tensor.matmul(
                            psum[:, 0 : hi - lo], bselb[:, :], m1[:, lo:hi],
                            start=(c == 0), stop=(c == NMM - 1),
                        )
                    pending.append((t, psum))
                    if len(pending) >= 3:
                        _emit_tail(*pending.pop(0))
                for pt, ppsum in pending:
                    _emit_tail(pt, ppsum)

            nc.sync.dma_start(chi_dram, chi[:, :])
            nc.sync.dma_start(acts_dram, acts[:, :])

    if legalize:
        _legalize_waits(nc)
    return nc


def _legalize_waits(nc, max_waits: int = 1):
    """This walrus build rejects instructions with more than one sync wait.
    Split excess waits onto preceding same-engine NoOps."""
    import concourse.mybir as mybir

    for f in nc.m.functions:
        for b in f.blocks:
            il = list(b.instructions)
            out, changed = [], False
            for inst in il:
                try:
                    si = inst.sync_info
                except AttributeError:
                    si = None
                waits = list(si.on_wait) if si else []
                if len(waits) > max_waits:
                    head, keep = waits[:-max_waits], waits[-max_waits:]
                    for k, wv in enumerate(head):
                        out.append(
                            mybir.InstNoOp(
                                name=f"{inst.name}-w{k}",
                                engine=inst.engine,
                                sync_info=mybir.SyncInfo(on_wait=[wv], on_update=[]),
                                bass_nofuse=True,
                            )
                        )
                    inst.sync_info = mybir.SyncInfo(
                        on_wait=keep, on_update=list(si.on_update)
                    )
                    changed = True
                out.append(inst)
            if changed:
                b.instructions = out


def make_host_inputs(xcore: np.ndarray):
    """xcore [SPC, H, W] f32 -> packed xi [NPART, WTOT]."""
    xi = np.full((SPC, 2, ROWS, SW), XSENT, dtype=np.float32)
    xh = xcore.reshape(SPC, 2, 64, W)
    xi[:, :, 2:66, 0:W] = xh
    xi[:, 1, 1, 0:W] = xcore[:, 63, :]   # h=1 up-overlap = image row 63
    xi[:, 0, 66, 0:W] = xcore[:, 64, :]  # h=0 down-overlap = image row 64
    return xi.reshape(NPART, WTOT)


def _host_bsel_bias():
    bsel = np.zeros((NPART, 64), dtype=np.float32)
    bsel[np.arange(NPART), np.arange(NPART) // 2] = 1.0
    bias = np.broadcast_to(
        -(np.arange(64, dtype=np.float32) + 0.5), (NPART, 64)
    ).copy()
    return bsel, bias


def _install_ntff_hook():
    import sys, types

    if "antenv.axon_hooks" in sys.modules:
        return
    mod = types.ModuleType("antenv.axon_hooks")
    state = {"hook": None}
    mod.set_axon_ntff_profile_hook = lambda h: state.update(hook=h)
    mod.get_axon_ntff_profile_hook = lambda: state["hook"]
    sys.modules["antenv.axon_hooks"] = mod
    try:
        from trn_agent_boot.trn_boot import _ntff_profile_via_ctypes

        hook = _ntff_profile_via_ctypes("/opt/axon/libaxon_pjrt.so")
        if hook is not None:
            mod.set_axon_ntff_profile_hook(hook)
    except Exception:
        pass


def _run(x: np.ndarray, trace: bool = False):
    from concourse import bass_utils

    if trace:
        _install_ntff_hook()

    x = np.ascontiguousarray(np.asarray(x), dtype=np.float32)
    assert x.shape == (B, C, H, W)

    if "nc" not in _CACHE:
        _CACHE["nc"] = _build_program()
    nc = _CACHE["nc"]

    bsel, bias = _host_bsel_bias()
    flat = x.reshape(SLICES, H, W)
    in_maps = []
    for k in range(NCORES):
        xi = make_host_inputs(flat[k * SPC : (k + 1) * SPC])
        in_maps.append({"xi": xi, "bsel": bsel, "bias": bias})
    res = bass_utils.run_bass_kernel_spmd(
        nc, in_maps, core_ids=list(range(NCORES)), trace=trace
    )

    ecc = np.empty((SLICES, RES), dtype=np.float64)
    for k in range(NCORES):
        chi = res.results[k]["chi"].astype(np.float64)    # [SPC, 64]
        acts = res.results[k]["acts"].astype(np.float64)  # [NPART, 4*ncell]
        a = acts.reshape(SPC, 2, len(CELL_TS), 4).sum(axis=1)
        sl = slice(k * SPC, (k + 1) * SPC)
        for t in range(RES - 1):
            src = _mask_source(t)
            if src == "ts":
                ecc[sl, t] = chi[:, t]
            elif src == "sign":
                # Sign mask: sum = Sw - 2*chi(t), Sw = sum(w) per slice = 1
                ecc[sl, t] = (1.0 - chi[:, t]) / 2.0
            else:
                ic = CELL_TS.index(t)
                # counts c = (N - a)/2 per array; widths cancel:
                # chi = (aEh + aEv - aV - aQ)/2
                ecc[sl, t] = (
                    a[:, ic, 1] + a[:, ic, 2] - a[:, ic, 0] - a[:, ic, 3]
                ) / 2.0
    ecc[:, RES - 1] = 1.0
    return ecc.reshape(B, C, RES).astype(np.float32), res


def kernel(x: np.ndarray) -> np.ndarray:
    out, _ = _run(x, trace=False)
    return out


# revision 14
# speedup vs baseline: 1.1927x; 1.0090x over previous
"""Euler characteristic curve (cubical complex) kernel for Trainium2.

Problem: x [32,16,128,128] f32 -> ECC [32,16,64] f32.
Per (b,c) slice: every cell of the 255x255 vertex-mode cubical grid has
filtration bin K = ceil(63*max(corner values)) in [0,63];
ECC(t) = #V(K<=t) - #Eh(K<=t) - #Ev(K<=t) + #Q(K<=t).

Strategy (per core, 64 slices, pure data parallel over 8 cores):
 - Lower-star compression: assign each cell to the lexicographically-first
   corner achieving its max bin; every cell assigned to vertex v activates at
   bin K_v, so chi(t) = sum_v w_v * [K_v <= t] with a t-independent integer
   weight w_v = 1 - (#edges assigned) + (#squares assigned) in [-3, 1].
   This turns 4 cell arrays into ONE weighted vertex array.
 - Layout: partition p = (slice s=p//2, half h=p%2); each partition holds 64
   image rows + up/down overlap rows, row stride 130 (128 cols + 2 sentinel
   cols; sentinel bin 1276 > 63 self-excludes from all counts).
 - Exact binning: y=63*x and the int-casts on ACT (any adjacent-integer cast
   rounding works for the ceil trick), compare+fix on DVE.
 - Steady state, per threshold: mask m0 = [K<=t] via DVE tensor_scalar @4x
   (some t) or ACT Sign (rest; engine balance), m1 = m0*w via DVE @2x, PE
   column-sum matmuls (constant blocksel weights) accumulate per-slice sums
   in PSUM, ACT Copy+accum_out tail-reduces PSUM -> chi[:, t] (sw-pipelined).
 - During the DVE-only weight-prep window, ACT runs a few thresholds in
   cell-mode instead: Sign+accum over the 4 plain cell arrays (V,Eh,Ev,Q).
 - All arithmetic is exact in integers; t = 63 is the Euler characteristic
   of the full square = 1 (host constant).
"""

import numpy as np

B, C, H, W = 32, 16, 128, 128
RES = 64
NCORES = 8
SLICES = B * C              # 512
SPC = SLICES // NCORES      # 64 slices per core
NPART = 128

SW = 130                    # row stride: 128 cols + 2 sentinel columns
ROWS = 67                   # pad row + up-overlap + 64 owned + down-overlap
WTOT = ROWS * SW            # 8710 input width per partition
KW = WTOT + 4               # K tile width (pad, memset to sentinel)
OWN = 260                   # owned rows start (flat offset, row 2)
OWN_W = 64 * SW             # 8320 owned width
EHX_W = 8582                # Ehx width (Eh over flat 129..8711)
XSENT = 20.25               # x-domain sentinel -> K = ceil(63*20.25) = 1276
NMM = 17                    # 16x512 + 1x128 moving chunks per threshold

CELL_TS = (15, 33, 53)      # thresholds counted in cell-mode during w-prep


def _mask_source(t: int) -> str:
    """'cell' (ACT 4-array count), 'ts' (DVE is_le), or 'sign' (ACT Sign)."""
    if t in CELL_TS:
        return "cell"
    return "ts" if (t % 3 == 1 or t == 0) else "sign"


_CACHE = {}


def _build_program(legalize=True):
    import concourse.bass as bass
    import concourse.mybir as mybir
    from concourse.tile import TileContext
    from concourse.alu_op_type import AluOpType as alu

    dt = mybir.dt
    af = mybir.ActivationFunctionType
    nc = bass.Bass("TRN2", target_bir_lowering=False, debug=False)

    x_dram = nc.dram_tensor("xi", [NPART, WTOT], dt.float32, kind="ExternalInput").ap()
    bsel_dram = nc.dram_tensor("bsel", [NPART, 64], dt.float32, kind="ExternalInput").ap()
    bias_dram = nc.dram_tensor("bias", [NPART, 64], dt.float32, kind="ExternalInput").ap()
    chi_dram = nc.dram_tensor("chi", [SPC, 64], dt.float32, kind="ExternalOutput").ap()
    acts_dram = nc.dram_tensor(
        "acts", [NPART, 4 * len(CELL_TS)], dt.float32, kind="ExternalOutput"
    ).ap()

    HCH = 2178  # K-compute column chunk width (4 chunks, last 2176)

    with TileContext(nc) as tc:
        with (
            tc.tile_pool(name="persist", bufs=1) as ap_,
            tc.tile_pool(name="ps", bufs=4, space="PSUM") as pp,
        ):
            # ---- persistent tiles ----
            K = ap_.tile([NPART, KW], dt.bfloat16)
            w = ap_.tile([NPART, OWN_W], dt.bfloat16)
            bself = ap_.tile([NPART, 64], dt.float32)
            bselb = ap_.tile([NPART, 64], dt.bfloat16)
            biasT = ap_.tile([NPART, 64], dt.float32)
            chi = ap_.tile([SPC, 64], dt.float32)
            acts = ap_.tile([NPART, 4 * len(CELL_TS)], dt.float32)
            scr512 = ap_.tile([SPC, 512], dt.bfloat16)

            nc.sync.dma_start(bself[:, :], bsel_dram)
            nc.sync.dma_start(biasT[:, :], bias_dram)
            nc.vector.tensor_copy(bselb[:, :], bself[:, :])
            nc.vector.memset(K[:, WTOT:KW], 1276.0)

            # ---- K = ceil(63*x): ACT does mult + casts, DVE compare + fix ----
            with tc.tile_pool(name="kprep", bufs=1) as kp:
                xf = kp.tile([NPART, WTOT], dt.float32)
                ft = kp.tile([NPART, 2 * HCH], dt.float32)
                it_ = kp.tile([NPART, HCH], dt.int32)
                ht = kp.tile([NPART, 2 * HCH], dt.bfloat16)
                chunks = [(i * HCH, min((i + 1) * HCH, WTOT)) for i in range(4)]
                for lo, hi in chunks:
                    nc.sync.dma_start(xf[:, lo:hi], x_dram[:, lo:hi])
                for lo, hi in chunks:
                    cw = hi - lo
                    y = ft[:, 0:cw]
                    yt = ft[:, HCH : HCH + cw]
                    ki = it_[:, 0:cw]
                    de = ht[:, 0:cw]
                    ytb = ht[:, HCH : HCH + cw]
                    nc.scalar.activation(y, xf[:, lo:hi], af.Copy, bias=0.0, scale=63.0)
                    nc.scalar.activation(ki, y, af.Copy)               # f32 -> int32
                    nc.scalar.activation(yt, ki, af.Copy)              # int32 -> f32
                    nc.scalar.activation(ytb, yt, af.Copy)             # f32 -> bf16
                    nc.vector.tensor_tensor(de, y, yt, alu.is_gt)      # 1x f32
                    nc.vector.tensor_tensor(K[:, lo:hi], de, ytb, alu.add)

            Ko = K[:, OWN : OWN + OWN_W]

            # ---- cell arrays + per-vertex weights; ACT runs cell-mode
            #      thresholds concurrently with the DVE-only weight prep ----
            with tc.tile_pool(name="wprep", bufs=1) as wp:
                ehx = wp.tile([NPART, EHX_W], dt.bfloat16)
                q = wp.tile([NPART, OWN_W], dt.bfloat16)
                ev = wp.tile([NPART, OWN_W], dt.bfloat16)
                ascr = wp.tile([NPART, OWN_W], dt.bfloat16)
                tt = wp.tile([NPART, 6 * OWN_W], dt.bfloat16)
                s = [tt[:, i * OWN_W : (i + 1) * OWN_W] for i in range(6)]
                TT = nc.vector.tensor_tensor

                TT(ehx[:, :], K[:, 129 : 129 + EHX_W], K[:, 130 : 130 + EHX_W], alu.max)
                TT(q[:, :], ehx[:, 131 : 131 + OWN_W], ehx[:, 261 : 261 + OWN_W], alu.max)
                TT(ev[:, :], Ko, K[:, OWN + SW : OWN + SW + OWN_W], alu.max)

                # cell-mode ACT thresholds (independent of w)
                cell_arrs = (Ko, ehx[:, 131 : 131 + OWN_W], ev[:, :], q[:, :])
                for ic, tc_ in enumerate(CELL_TS):
                    for j, arr in enumerate(cell_arrs):
                        nc.scalar.activation(
                            ascr[:, :], arr, af.Sign,
                            bias=biasT[:, tc_ : tc_ + 1], scale=1.0,
                            accum_out=acts[:, 4 * ic + j : 4 * ic + j + 1],
                        )

                TT(s[0], Ko, K[:, OWN - 1 : OWN - 1 + OWN_W], alu.is_gt)    # bL'
                TT(s[1], Ko, K[:, OWN + 1 : OWN + 1 + OWN_W], alu.is_ge)    # bR'
                TT(s[2], Ko, K[:, OWN - SW : OWN - SW + OWN_W], alu.is_gt)  # bU'
                TT(s[3], Ko, K[:, OWN + SW : OWN + SW + OWN_W], alu.is_ge)  # bD'
                TT(s[4], s[0], s[1], alu.add)                               # e1
                TT(s[5], s[2], s[3], alu.add)                               # e2
                TT(s[2], s[4], s[5], alu.add)                               # E
                TT(s[3], Ko, ehx[:, 0:OWN_W], alu.is_gt)                    # cUL
                TT(s[4], s[3], s[0], alu.mult)                              # S_ul
                TT(s[5], Ko, ehx[:, 1 : 1 + OWN_W], alu.is_gt)              # cUR
                TT(s[3], s[5], s[1], alu.mult)                              # S_ur
                TT(s[5], Ko, ehx[:, 260 : 260 + OWN_W], alu.is_ge)          # cLL
                TT(s[1], s[5], s[0], alu.mult)                              # S_ll
                TT(s[5], Ko, q[:, :], alu.is_ge)                            # S_lr
                TT(s[0], s[4], s[3], alu.add)                               # S_ul+S_ur
                TT(s[3], s[1], s[5], alu.add)                               # S_ll+S_lr
                TT(s[4], s[0], s[3], alu.add)                               # S
                nc.vector.tensor_scalar(s[5], s[2], -1.0, 1.0, alu.mult, alu.add)  # 1-E
                TT(s[1], s[4], s[5], alu.add)                               # w (raw)
                # zero w at sentinel positions so sign-sourced sums are clean
                nc.vector.tensor_scalar(s[0], Ko, 63.0, 1.0, alu.is_le, alu.mult)
                TT(w[:, :], s[1], s[0], alu.mult)                           # w

            # ---- threshold loop (mask*w -> PE colsum -> ACT tail) ----
            # sign-sourced decode: sum = Sw - 2*chi(t) with Sw = sum(w) = 1
            with tc.tile_pool(name="thr", bufs=3) as mp:
                pending = []

                def _emit_tail(pt, ppsum):
                    if pt % 16 == 2:
                        nc.vector.tensor_reduce(
                            chi[:, pt : pt + 1], ppsum[:, :],
                            mybir.AxisListType.X, alu.add,
                        )
                    else:
                        nc.scalar.activation(
                            scr512[:, :], ppsum[:, :], af.Copy, bias=0.0, scale=1.0,
                            accum_out=chi[:, pt : pt + 1],
                        )

                for t in range(RES - 1):
                    src = _mask_source(t)
                    if src == "cell":
                        continue
                    m0 = mp.tile([NPART, OWN_W], dt.bfloat16, tag="m0", bufs=4)
                    m1 = mp.tile([NPART, OWN_W], dt.bfloat16, tag="m1", bufs=4)
                    if src == "ts":
                        nc.vector.tensor_scalar(m0[:, :], Ko, float(t), None, alu.is_le)
                    else:
                        nc.scalar.activation(
                            m0[:, :], Ko, af.Sign,
                            bias=biasT[:, t : t + 1], scale=1.0,
                        )
                    nc.vector.tensor_tensor(m1[:, :], m0[:, :], w[:, :], alu.mult)
                    psum = pp.tile([SPC, 512], dt.float32, tag="ps")
                    for c in range(NMM):
                        lo = 512 * c
                        hi = min(512 * (c + 1), OWN_W)
                        nc.tensor.matmul(
                            psum[:, 0 : hi - lo], bselb[:, :], m1[:, lo:hi],
                            start=(c == 0), stop=(c == NMM - 1),
                        )
                    pending.append((t, psum))
                    if len(pending) >= 3:
                        _emit_tail(*pending.pop(0))
                for pt, ppsum in pending:
                    _emit_tail(pt, ppsum)

            nc.sync.dma_start(chi_dram, chi[:, :])
            nc.sync.dma_start(acts_dram, acts[:, :])

    if legalize:
        _legalize_waits(nc)
    return nc


def _legalize_waits(nc, max_waits: int = 1):
    """This walrus build rejects instructions with more than one sync wait.
    Split excess waits onto preceding same-engine NoOps."""
    import concourse.mybir as mybir

    for f in nc.m.functions:
        for b in f.blocks:
            il = list(b.instructions)
            out, changed = [], False
            for inst in il:
                try:
                    si = inst.sync_info
                except AttributeError:
                    si = None
                waits = list(si.on_wait) if si else []
                if len(waits) > max_waits:
                    head, keep = waits[:-max_waits], waits[-max_waits:]
                    for k, wv in enumerate(head):
                        out.append(
                            mybir.InstNoOp(
                                name=f"{inst.name}-w{k}",
                                engine=inst.engine,
                                sync_info=mybir.SyncInfo(on_wait=[wv], on_update=[]),
                                bass_nofuse=True,
                            )
                        )
                    inst.sync_info = mybir.SyncInfo(
                        on_wait=keep, on_update=list(si.on_update)
                    )
                    changed = True
                out.append(inst)
            if changed:
                b.instructions = out


def make_host_inputs(xcore: np.ndarray):
    """xcore [SPC, H, W] f32 -> packed xi [NPART, WTOT]."""
    xi = np.full((SPC, 2, ROWS, SW), XSENT, dtype=np.float32)
    xh = xcore.reshape(SPC, 2, 64, W)
    xi[:, :, 2:66, 0:W] = xh
    xi[:, 1, 1, 0:W] = xcore[:, 63, :]   # h=1 up-overlap = image row 63
    xi[:, 0, 66, 0:W] = xcore[:, 64, :]  # h=0 down-overlap = image row 64
    return xi.reshape(NPART, WTOT)


def _host_bsel_bias():
    bsel = np.zeros((NPART, 64), dtype=np.float32)
    bsel[np.arange(NPART), np.arange(NPART) // 2] = 1.0
    bias = np.broadcast_to(
        -(np.arange(64, dtype=np.float32) + 0.5), (NPART, 64)
    ).copy()
    return bsel, bias


def _install_ntff_hook():
    import sys, types

    if "antenv.axon_hooks" in sys.modules:
        return
    mod = types.ModuleType("antenv.axon_hooks")
    state = {"hook": None}
    mod.set_axon_ntff_profile_hook = lambda h: state.update(hook=h)
    mod.get_axon_ntff_profile_hook = lambda: state["hook"]
    sys.modules["antenv.axon_hooks"] = mod
    try:
        from trn_agent_boot.trn_boot import _ntff_profile_via_ctypes

        hook = _ntff_profile_via_ctypes("/opt/axon/libaxon_pjrt.so")
        if hook is not None:
            mod.set_axon_ntff_profile_hook(hook)
    except Exception:
        pass


def _run(x: np.ndarray, trace: bool = False):
    from concourse import bass_utils

    if trace:
        _install_ntff_hook()

    x = np.ascontiguousarray(np.asarray(x), dtype=np.float32)
    assert x.shape == (B, C, H, W)

    if "nc" not in _CACHE:
        _CACHE["nc"] = _build_program()
    nc = _CACHE["nc"]

    bsel, bias = _host_bsel_bias()
    flat = x.reshape(SLICES, H, W)
    in_maps = []
    for k in range(NCORES):
        xi = make_host_inputs(flat[k * SPC : (k + 1) * SPC])
        in_maps.append({"xi": xi, "bsel": bsel, "bias": bias})
    res = bass_utils.run_bass_kernel_spmd(
        nc, in_maps, core_ids=list(range(NCORES)), trace=trace
    )

    ecc = np.empty((SLICES, RES), dtype=np.float64)
    for k in range(NCORES):
        chi = res.results[k]["chi"].astype(np.float64)    # [SPC, 64]
        acts = res.results[k]["acts"].astype(np.float64)  # [NPART, 4*ncell]
        a = acts.reshape(SPC, 2, len(CELL_TS), 4).sum(axis=1)
        sl = slice(k * SPC, (k + 1) * SPC)
        for t in range(RES - 1):
            src = _mask_source(t)
            if src == "ts":
                ecc[sl, t] = chi[:, t]
            elif src == "sign":
                # Sign mask: sum = Sw - 2*chi(t), Sw = sum(w) per slice = 1
                ecc[sl, t] = (1.0 - chi[:, t]) / 2.0
            else:
                ic = CELL_TS.index(t)
                # counts c = (N - a)/2 per array; widths cancel:
                # chi = (aEh + aEv - aV - aQ)/2
                ecc[sl, t] = (
                    a[:, ic, 1] + a[:, ic, 2] - a[:, ic, 0] - a[:, ic, 3]
                ) / 2.0
    ecc[:, RES - 1] = 1.0
    return ecc.reshape(B, C, RES).astype(np.float32), res


def kernel(x: np.ndarray) -> np.ndarray:
    out, _ = _run(x, trace=False)
    return out


# revision 17
# speedup vs baseline: 1.1987x; 1.0050x over previous
"""Euler characteristic curve (cubical complex) kernel for Trainium2.

Problem: x [32,16,128,128] f32 -> ECC [32,16,64] f32.
Per (b,c) slice: every cell of the 255x255 vertex-mode cubical grid has
filtration bin K = ceil(63*max(corner values)) in [0,63];
ECC(t) = #V(K<=t) - #Eh(K<=t) - #Ev(K<=t) + #Q(K<=t).

Strategy (per core, 64 slices, pure data parallel over 8 cores):
 - Lower-star compression: assign each cell to the lexicographically-first
   corner achieving its max bin; every cell assigned to vertex v activates at
   bin K_v, so chi(t) = sum_v w_v * [K_v <= t] with a t-independent integer
   weight w_v = 1 - (#edges assigned) + (#squares assigned) in [-3, 1].
   This turns 4 cell arrays into ONE weighted vertex array.
 - Layout: partition p = (slice s=p//2, half h=p%2); each partition holds 64
   image rows + up/down overlap rows, row stride 130 (128 cols + 2 sentinel
   cols; sentinel bin 1276 > 63 self-excludes from all counts).
 - Exact binning: y=63*x and the int-casts on ACT (any adjacent-integer cast
   rounding works for the ceil trick), compare+fix on DVE.
 - Steady state, per threshold: mask m0 = [K<=t] via DVE tensor_scalar @4x
   (some t) or ACT Sign (rest; engine balance), m1 = m0*w via DVE @2x, PE
   column-sum matmuls (constant blocksel weights) accumulate per-slice sums
   in PSUM, ACT Copy+accum_out tail-reduces PSUM -> chi[:, t] (sw-pipelined).
 - During the DVE-only weight-prep window, ACT runs a few thresholds in
   cell-mode instead: Sign+accum over the 4 plain cell arrays (V,Eh,Ev,Q).
 - All arithmetic is exact in integers; t = 63 is the Euler characteristic
   of the full square = 1 (host constant).
"""

import numpy as np

B, C, H, W = 32, 16, 128, 128
RES = 64
NCORES = 8
SLICES = B * C              # 512
SPC = SLICES // NCORES      # 64 slices per core
NPART = 128

SW = 130                    # row stride: 128 cols + 2 sentinel columns
ROWS = 67                   # pad row + up-overlap + 64 owned + down-overlap
WTOT = ROWS * SW            # 8710 input width per partition
KW = WTOT + 4               # K tile width (pad, memset to sentinel)
OWN = 260                   # owned rows start (flat offset, row 2)
OWN_W = 64 * SW             # 8320 owned width
EHX_W = 8582                # Ehx width (Eh over flat 129..8711)
XSENT = 20.25               # x-domain sentinel -> K = ceil(63*20.25) = 1276
NMM = 17                    # 16x512 + 1x128 moving chunks per threshold

CELL_TS = (15, 33, 53)      # thresholds counted in cell-mode during w-prep
XSPL = 4992                 # mask split point: ACT Sign on [0:XSPL], DVE is_le rest


_CACHE = {}


def _build_program(legalize=True):
    import concourse.bass as bass
    import concourse.mybir as mybir
    from concourse.tile import TileContext
    from concourse.alu_op_type import AluOpType as alu

    dt = mybir.dt
    af = mybir.ActivationFunctionType
    nc = bass.Bass("TRN2", target_bir_lowering=False, debug=False)

    x_dram = nc.dram_tensor("xi", [NPART, WTOT], dt.float32, kind="ExternalInput").ap()
    bsel_dram = nc.dram_tensor("bsel", [NPART, 64], dt.float32, kind="ExternalInput").ap()
    bias_dram = nc.dram_tensor("bias", [NPART, 64], dt.float32, kind="ExternalInput").ap()
    chi_dram = nc.dram_tensor("chi", [SPC, 64], dt.float32, kind="ExternalOutput").ap()
    acts_dram = nc.dram_tensor(
        "acts", [NPART, 4 * len(CELL_TS)], dt.float32, kind="ExternalOutput"
    ).ap()

    HCH = 2178  # K-compute column chunk width (4 chunks, last 2176)

    with TileContext(nc) as tc:
        with (
            tc.tile_pool(name="persist", bufs=1) as ap_,
            tc.tile_pool(name="ps", bufs=4, space="PSUM") as pp,
        ):
            # ---- persistent tiles ----
            K = ap_.tile([NPART, KW], dt.bfloat16)
            w = ap_.tile([NPART, OWN_W], dt.bfloat16)
            w2 = ap_.tile([NPART, OWN_W], dt.bfloat16)
            bself = ap_.tile([NPART, 64], dt.float32)
            bselb = ap_.tile([NPART, 64], dt.bfloat16)
            biasT = ap_.tile([NPART, 64], dt.float32)
            chi = ap_.tile([SPC, 64], dt.float32)
            acts = ap_.tile([NPART, 4 * len(CELL_TS)], dt.float32)
            scr512 = ap_.tile([SPC, 512], dt.bfloat16)

            nc.sync.dma_start(bself[:, :], bsel_dram)
            nc.sync.dma_start(biasT[:, :], bias_dram)
            nc.vector.tensor_copy(bselb[:, :], bself[:, :])
            nc.vector.memset(K[:, WTOT:KW], 1276.0)

            # ---- K = ceil(63*x): ACT does mult + casts, DVE compare + fix ----
            with tc.tile_pool(name="kprep", bufs=1) as kp:
                xf = kp.tile([NPART, WTOT], dt.float32)
                ft = kp.tile([NPART, 2 * HCH], dt.float32)
                it_ = kp.tile([NPART, HCH], dt.int32)
                ht = kp.tile([NPART, 2 * HCH], dt.bfloat16)
                chunks = [(i * HCH, min((i + 1) * HCH, WTOT)) for i in range(4)]
                for lo, hi in chunks:
                    nc.sync.dma_start(xf[:, lo:hi], x_dram[:, lo:hi])
                for lo, hi in chunks:
                    cw = hi - lo
                    y = ft[:, 0:cw]
                    yt = ft[:, HCH : HCH + cw]
                    ki = it_[:, 0:cw]
                    de = ht[:, 0:cw]
                    ytb = ht[:, HCH : HCH + cw]
                    nc.scalar.activation(y, xf[:, lo:hi], af.Copy, bias=0.0, scale=63.0)
                    nc.scalar.activation(ki, y, af.Copy)               # f32 -> int32
                    nc.scalar.activation(yt, ki, af.Copy)              # int32 -> f32
                    nc.scalar.activation(ytb, yt, af.Copy)             # f32 -> bf16
                    nc.vector.tensor_tensor(de, y, yt, alu.is_gt)      # 1x f32
                    nc.vector.tensor_tensor(K[:, lo:hi], de, ytb, alu.add)

            Ko = K[:, OWN : OWN + OWN_W]

            # ---- cell arrays + per-vertex weights; ACT runs cell-mode
            #      thresholds concurrently with the DVE-only weight prep ----
            with tc.tile_pool(name="wprep", bufs=1) as wp:
                ehx = wp.tile([NPART, EHX_W], dt.bfloat16)
                q = wp.tile([NPART, OWN_W], dt.bfloat16)
                ev = wp.tile([NPART, OWN_W], dt.bfloat16)
                ascr = wp.tile([NPART, OWN_W], dt.float8e4)
                tt = wp.tile([NPART, 6 * OWN_W], dt.bfloat16)
                s = [tt[:, i * OWN_W : (i + 1) * OWN_W] for i in range(6)]
                TT = nc.vector.tensor_tensor

                TT(ehx[:, :], K[:, 129 : 129 + EHX_W], K[:, 130 : 130 + EHX_W], alu.max)
                TT(q[:, :], ehx[:, 131 : 131 + OWN_W], ehx[:, 261 : 261 + OWN_W], alu.max)
                TT(ev[:, :], Ko, K[:, OWN + SW : OWN + SW + OWN_W], alu.max)

                # cell-mode ACT thresholds (independent of w)
                cell_arrs = (Ko, ehx[:, 131 : 131 + OWN_W], ev[:, :], q[:, :])
                for ic, tc_ in enumerate(CELL_TS):
                    for j, arr in enumerate(cell_arrs):
                        nc.scalar.activation(
                            ascr[:, :], arr, af.Sign,
                            bias=biasT[:, tc_ : tc_ + 1], scale=1.0,
                            accum_out=acts[:, 4 * ic + j : 4 * ic + j + 1],
                        )

                TT(s[0], Ko, K[:, OWN - 1 : OWN - 1 + OWN_W], alu.is_gt)    # bL'
                TT(s[1], Ko, K[:, OWN + 1 : OWN + 1 + OWN_W], alu.is_ge)    # bR'
                TT(s[2], Ko, K[:, OWN - SW : OWN - SW + OWN_W], alu.is_gt)  # bU'
                TT(s[3], Ko, K[:, OWN + SW : OWN + SW + OWN_W], alu.is_ge)  # bD'
                TT(s[4], s[0], s[1], alu.add)                               # e1
                TT(s[5], s[2], s[3], alu.add)                               # e2
                TT(s[2], s[4], s[5], alu.add)                               # E
                TT(s[3], Ko, ehx[:, 0:OWN_W], alu.is_gt)                    # cUL
                TT(s[4], s[3], s[0], alu.mult)                              # S_ul
                TT(s[5], Ko, ehx[:, 1 : 1 + OWN_W], alu.is_gt)              # cUR
                TT(s[3], s[5], s[1], alu.mult)                              # S_ur
                TT(s[5], Ko, ehx[:, 260 : 260 + OWN_W], alu.is_ge)          # cLL
                TT(s[1], s[5], s[0], alu.mult)                              # S_ll
                TT(s[5], Ko, q[:, :], alu.is_ge)                            # S_lr
                TT(s[0], s[4], s[3], alu.add)                               # S_ul+S_ur
                TT(s[3], s[1], s[5], alu.add)                               # S_ll+S_lr
                TT(s[4], s[0], s[3], alu.add)                               # S
                nc.vector.tensor_scalar(s[5], s[2], -1.0, 1.0, alu.mult, alu.add)  # 1-E
                TT(s[1], s[4], s[5], alu.add)                               # w (raw)
                # zero w at sentinel positions so sign-sourced sums are clean
                nc.vector.tensor_scalar(s[0], Ko, 63.0, 1.0, alu.is_le, alu.mult)
                TT(w[:, :], s[1], s[0], alu.mult)                           # w

            # w2: -w/2 on the ACT-Sign half, w on the DVE-is_le half.
            # Sign half sums give sum(-w/2 * s_t) = chiA(t) - SwA/2, so
            # chi(t) = psum_total(t) - C with C = sum_A(w2) (chi col 63).
            nc.vector.tensor_scalar(w2[:, 0:XSPL], w[:, 0:XSPL], -0.5, None, alu.mult)
            nc.vector.tensor_copy(w2[:, XSPL:OWN_W], w[:, XSPL:OWN_W])

            # ---- threshold loop (mask*w -> PE colsum -> ACT tail) ----
            # sign-sourced decode: sum = Sw - 2*chi(t) with Sw = sum(w) = 1
            with tc.tile_pool(name="thr", bufs=3) as mp:
                pending = []

                def _emit_tail(pt, ppsum):
                    if pt % 16 == 2:
                        nc.vector.tensor_reduce(
                            chi[:, pt : pt + 1], ppsum[:, :],
                            mybir.AxisListType.X, alu.add,
                        )
                    else:
                        nc.scalar.activation(
                            scr512[:, :], ppsum[:, :], af.Copy, bias=0.0, scale=1.0,
                            accum_out=chi[:, pt : pt + 1],
                        )

                # C = sum_A(w2) per slice -> chi[:, 63] (decode constant)
                cpsum = pp.tile([SPC, 512], dt.float32, tag="ps")
                for c in range((XSPL + 511) // 512):
                    lo = 512 * c
                    hi = min(512 * (c + 1), XSPL)
                    nc.tensor.matmul(
                        cpsum[:, 0 : hi - lo], bselb[:, :], w2[:, lo:hi],
                        start=(c == 0), stop=(hi == XSPL),
                    )
                pending.append((RES - 1, cpsum))

                for t in range(RES - 1):
                    if t in CELL_TS:
                        continue
                    m0a = mp.tile([NPART, XSPL], dt.bfloat16, tag="m0a", bufs=4)
                    m0b = mp.tile([NPART, OWN_W - XSPL], dt.bfloat16, tag="m0b", bufs=4)
                    m1 = mp.tile([NPART, OWN_W], dt.bfloat16, tag="m1", bufs=4)
                    nc.scalar.activation(
                        m0a[:, :], K[:, OWN : OWN + XSPL], af.Sign,
                        bias=biasT[:, t : t + 1], scale=1.0,
                    )
                    nc.vector.tensor_scalar(
                        m0b[:, :], K[:, OWN + XSPL : OWN + OWN_W], float(t),
                        None, alu.is_le,
                    )
                    nc.vector.tensor_tensor(
                        m1[:, 0:XSPL], m0a[:, :], w2[:, 0:XSPL], alu.mult
                    )
                    nc.vector.tensor_tensor(
                        m1[:, XSPL:OWN_W], m0b[:, :], w2[:, XSPL:OWN_W], alu.mult
                    )
                    psum = pp.tile([SPC, 512], dt.float32, tag="ps")
                    for c in range(NMM):
                        lo = 512 * c
                        hi = min(512 * (c + 1), OWN_W)
                        nc.tensor.matmul(
                            psum[:, 0 : hi - lo], bselb[:, :], m1[:, lo:hi],
                            start=(c == 0), stop=(c == NMM - 1),
                        )
                    pending.append((t, psum))
                    if len(pending) >= 3:
                        _emit_tail(*pending.pop(0))
                for pt, ppsum in pending:
                    _emit_tail(pt, ppsum)

            nc.sync.dma_start(chi_dram, chi[:, :])
            nc.sync.dma_start(acts_dram, acts[:, :])

    if legalize:
        _legalize_waits(nc)
    return nc


def _legalize_waits(nc, max_waits: int = 1):
    """This walrus build rejects instructions with more than one sync wait.
    Split excess waits onto preceding same-engine NoOps."""
    import concourse.mybir as mybir

    for f in nc.m.functions:
        for b in f.blocks:
            il = list(b.instructions)
            out, changed = [], False
            for inst in il:
                try:
                    si = inst.sync_info
                except AttributeError:
                    si = None
                waits = list(si.on_wait) if si else []
                if len(waits) > max_waits:
                    head, keep = waits[:-max_waits], waits[-max_waits:]
                    for k, wv in enumerate(head):
                        out.append(
                            mybir.InstNoOp(
                                name=f"{inst.name}-w{k}",
                                engine=inst.engine,
                                sync_info=mybir.SyncInfo(on_wait=[wv], on_update=[]),
                                bass_nofuse=True,
                            )
                        )
                    inst.sync_info = mybir.SyncInfo(
                        on_wait=keep, on_update=list(si.on_update)
                    )
                    changed = True
                out.append(inst)
            if changed:
                b.instructions = out


def make_host_inputs(xcore: np.ndarray):
    """xcore [SPC, H, W] f32 -> packed xi [NPART, WTOT]."""
    xi = np.full((SPC, 2, ROWS, SW), XSENT, dtype=np.float32)
    xh = xcore.reshape(SPC, 2, 64, W)
    xi[:, :, 2:66, 0:W] = xh
    xi[:, 1, 1, 0:W] = xcore[:, 63, :]   # h=1 up-overlap = image row 63
    xi[:, 0, 66, 0:W] = xcore[:, 64, :]  # h=0 down-overlap = image row 64
    return xi.reshape(NPART, WTOT)


def _host_bsel_bias():
    bsel = np.zeros((NPART, 64), dtype=np.float32)
    bsel[np.arange(NPART), np.arange(NPART) // 2] = 1.0
    bias = np.broadcast_to(
        -(np.arange(64, dtype=np.float32) + 0.5), (NPART, 64)
    ).copy()
    return bsel, bias


def _install_ntff_hook():
    import sys, types

    if "antenv.axon_hooks" in sys.modules:
        return
    mod = types.ModuleType("antenv.axon_hooks")
    state = {"hook": None}
    mod.set_axon_ntff_profile_hook = lambda h: state.update(hook=h)
    mod.get_axon_ntff_profile_hook = lambda: state["hook"]
    sys.modules["antenv.axon_hooks"] = mod
    try:
        from trn_agent_boot.trn_boot import _ntff_profile_via_ctypes

        hook = _ntff_profile_via_ctypes("/opt/axon/libaxon_pjrt.so")
        if hook is not None:
            mod.set_axon_ntff_profile_hook(hook)
    except Exception:
        pass


def _run(x: np.ndarray, trace: bool = False):
    from concourse import bass_utils

    if trace:
        _install_ntff_hook()

    x = np.ascontiguousarray(np.asarray(x), dtype=np.float32)
    assert x.shape == (B, C, H, W)

    if "nc" not in _CACHE:
        _CACHE["nc"] = _build_program()
    nc = _CACHE["nc"]

    bsel, bias = _host_bsel_bias()
    flat = x.reshape(SLICES, H, W)
    in_maps = []
    for k in range(NCORES):
        xi = make_host_inputs(flat[k * SPC : (k + 1) * SPC])
        in_maps.append({"xi": xi, "bsel": bsel, "bias": bias})
    res = bass_utils.run_bass_kernel_spmd(
        nc, in_maps, core_ids=list(range(NCORES)), trace=trace
    )

    ecc = np.empty((SLICES, RES), dtype=np.float64)
    for k in range(NCORES):
        chi = res.results[k]["chi"].astype(np.float64)    # [SPC, 64]
        acts = res.results[k]["acts"].astype(np.float64)  # [NPART, 4*ncell]
        a = acts.reshape(SPC, 2, len(CELL_TS), 4).sum(axis=1)
        sl = slice(k * SPC, (k + 1) * SPC)
        for t in range(RES - 1):
            if t in CELL_TS:
                ic = CELL_TS.index(t)
                # counts c = (N - a)/2 per array; widths cancel:
                # chi = (aEh + aEv - aV - aQ)/2
                ecc[sl, t] = (
                    a[:, ic, 1] + a[:, ic, 2] - a[:, ic, 0] - a[:, ic, 3]
                ) / 2.0
            else:
                # split mask: psum total = chi(t) + C, C stored in col 63
                ecc[sl, t] = chi[:, t] - chi[:, 63]
    ecc[:, RES - 1] = 1.0
    return ecc.reshape(B, C, RES).astype(np.float32), res


def kernel(x: np.ndarray) -> np.ndarray:
    out, _ = _run(x, trace=False)
    return out


# revision 18
# speedup vs baseline: 1.2144x; 1.0131x over previous
"""Euler characteristic curve (cubical complex) kernel for Trainium2.

Problem: x [32,16,128,128] f32 -> ECC [32,16,64] f32.
Per (b,c) slice: every cell of the 255x255 vertex-mode cubical grid has
filtration bin K = ceil(63*max(corner values)) in [0,63];
ECC(t) = #V(K<=t) - #Eh(K<=t) - #Ev(K<=t) + #Q(K<=t).

Strategy (per core, 64 slices, pure data parallel over 8 cores):
 - Lower-star compression: assign each cell to the lexicographically-first
   corner achieving its max bin; every cell assigned to vertex v activates at
   bin K_v, so chi(t) = sum_v w_v * [K_v <= t] with a t-independent integer
   weight w_v = 1 - (#edges assigned) + (#squares assigned) in [-3, 1].
   This turns 4 cell arrays into ONE weighted vertex array.
 - Layout: partition p = (slice s=p//2, half h=p%2); each partition holds 64
   image rows + up/down overlap rows, row stride 130 (128 cols + 2 sentinel
   cols; sentinel bin 1276 > 63 self-excludes from all counts).
 - Exact binning: y=63*x and the int-casts on ACT (any adjacent-integer cast
   rounding works for the ceil trick), compare+fix on DVE.
 - Steady state, per threshold: mask m0 = [K<=t] via DVE tensor_scalar @4x
   (some t) or ACT Sign (rest; engine balance), m1 = m0*w via DVE @2x, PE
   column-sum matmuls (constant blocksel weights) accumulate per-slice sums
   in PSUM, ACT Copy+accum_out tail-reduces PSUM -> chi[:, t] (sw-pipelined).
 - During the DVE-only weight-prep window, ACT runs a few thresholds in
   cell-mode instead: Sign+accum over the 4 plain cell arrays (V,Eh,Ev,Q).
 - All arithmetic is exact in integers; t = 63 is the Euler characteristic
   of the full square = 1 (host constant).
"""

import numpy as np

B, C, H, W = 32, 16, 128, 128
RES = 64
NCORES = 8
SLICES = B * C              # 512
SPC = SLICES // NCORES      # 64 slices per core
NPART = 128

SW = 130                    # row stride: 128 cols + 2 sentinel columns
ROWS = 67                   # pad row + up-overlap + 64 owned + down-overlap
WTOT = ROWS * SW            # 8710 input width per partition
KW = WTOT + 4               # K tile width (pad, memset to sentinel)
OWN = 260                   # owned rows start (flat offset, row 2)
OWN_W = 64 * SW             # 8320 owned width
EHX_W = 8582                # Ehx width (Eh over flat 129..8711)
XSENT = 20.25               # x-domain sentinel -> K = ceil(63*20.25) = 1276
NMM = 17                    # 16x512 + 1x128 moving chunks per threshold

CELL_TS = (15, 33, 53)      # thresholds counted in cell-mode during w-prep
XSPL = 5184                 # mask split point: ACT Sign on [0:XSPL], DVE is_le rest


_CACHE = {}


def _build_program(legalize=True):
    import concourse.bass as bass
    import concourse.mybir as mybir
    from concourse.tile import TileContext
    from concourse.alu_op_type import AluOpType as alu

    dt = mybir.dt
    af = mybir.ActivationFunctionType
    nc = bass.Bass("TRN2", target_bir_lowering=False, debug=False)

    x_dram = nc.dram_tensor("xi", [NPART, WTOT], dt.float32, kind="ExternalInput").ap()
    bsel_dram = nc.dram_tensor("bsel", [NPART, 64], dt.float32, kind="ExternalInput").ap()
    bias_dram = nc.dram_tensor("bias", [NPART, 64], dt.float32, kind="ExternalInput").ap()
    chi_dram = nc.dram_tensor("chi", [SPC, 64], dt.float32, kind="ExternalOutput").ap()
    acts_dram = nc.dram_tensor(
        "acts", [NPART, 4 * len(CELL_TS)], dt.float32, kind="ExternalOutput"
    ).ap()

    HCH = 2178  # K-compute column chunk width (4 chunks, last 2176)

    with TileContext(nc) as tc:
        with (
            tc.tile_pool(name="persist", bufs=1) as ap_,
            tc.tile_pool(name="ps", bufs=4, space="PSUM") as pp,
        ):
            # ---- persistent tiles ----
            K = ap_.tile([NPART, KW], dt.bfloat16)
            w = ap_.tile([NPART, OWN_W], dt.bfloat16)
            w2 = ap_.tile([NPART, OWN_W], dt.bfloat16)
            bself = ap_.tile([NPART, 64], dt.float32)
            bselb = ap_.tile([NPART, 64], dt.bfloat16)
            biasT = ap_.tile([NPART, 64], dt.float32)
            chi = ap_.tile([SPC, 64], dt.float32)
            acts = ap_.tile([NPART, 4 * len(CELL_TS)], dt.float32)
            scr512 = ap_.tile([SPC, 512], dt.bfloat16)

            nc.sync.dma_start(bself[:, :], bsel_dram)
            nc.sync.dma_start(biasT[:, :], bias_dram)
            nc.vector.tensor_copy(bselb[:, :], bself[:, :])
            nc.vector.memset(K[:, WTOT:KW], 1276.0)

            # ---- K = ceil(63*x): ACT does mult + casts, DVE compare + fix ----
            with tc.tile_pool(name="kprep", bufs=1) as kp:
                xf = kp.tile([NPART, WTOT], dt.float32)
                ft = kp.tile([NPART, 2 * HCH], dt.float32)
                it_ = kp.tile([NPART, HCH], dt.int32)
                ht = kp.tile([NPART, 2 * HCH], dt.bfloat16)
                chunks = [(i * HCH, min((i + 1) * HCH, WTOT)) for i in range(4)]
                for lo, hi in chunks:
                    nc.sync.dma_start(xf[:, lo:hi], x_dram[:, lo:hi])
                for lo, hi in chunks:
                    cw = hi - lo
                    y = ft[:, 0:cw]
                    yt = ft[:, HCH : HCH + cw]
                    ki = it_[:, 0:cw]
                    de = ht[:, 0:cw]
                    ytb = ht[:, HCH : HCH + cw]
                    nc.scalar.activation(y, xf[:, lo:hi], af.Copy, bias=0.0, scale=63.0)
                    nc.scalar.activation(ki, y, af.Copy)               # f32 -> int32
                    nc.scalar.activation(yt, ki, af.Copy)              # int32 -> f32
                    nc.scalar.activation(ytb, yt, af.Copy)             # f32 -> bf16
                    nc.vector.tensor_tensor(de, y, yt, alu.is_gt)      # 1x f32
                    nc.vector.tensor_tensor(K[:, lo:hi], de, ytb, alu.add)

            Ko = K[:, OWN : OWN + OWN_W]

            # ---- cell arrays + per-vertex weights; ACT runs cell-mode
            #      thresholds concurrently with the DVE-only weight prep ----
            with tc.tile_pool(name="wprep", bufs=1) as wp:
                ehx = wp.tile([NPART, EHX_W], dt.bfloat16)
                q = wp.tile([NPART, OWN_W], dt.bfloat16)
                ev = wp.tile([NPART, OWN_W], dt.bfloat16)
                ascr = wp.tile([NPART, OWN_W], dt.float8e4)
                tt = wp.tile([NPART, 6 * OWN_W], dt.bfloat16)
                s = [tt[:, i * OWN_W : (i + 1) * OWN_W] for i in range(6)]
                TT = nc.vector.tensor_tensor

                TT(ehx[:, :], K[:, 129 : 129 + EHX_W], K[:, 130 : 130 + EHX_W], alu.max)
                TT(q[:, :], ehx[:, 131 : 131 + OWN_W], ehx[:, 261 : 261 + OWN_W], alu.max)
                TT(ev[:, :], Ko, K[:, OWN + SW : OWN + SW + OWN_W], alu.max)

                # cell-mode ACT thresholds (independent of w)
                cell_arrs = (Ko, ehx[:, 131 : 131 + OWN_W], ev[:, :], q[:, :])
                for ic, tc_ in enumerate(CELL_TS):
                    for j, arr in enumerate(cell_arrs):
                        nc.scalar.activation(
                            ascr[:, :], arr, af.Sign,
                            bias=biasT[:, tc_ : tc_ + 1], scale=1.0,
                            accum_out=acts[:, 4 * ic + j : 4 * ic + j + 1],
                        )

                TT(s[0], Ko, K[:, OWN - 1 : OWN - 1 + OWN_W], alu.is_gt)    # bL'
                TT(s[1], Ko, K[:, OWN + 1 : OWN + 1 + OWN_W], alu.is_ge)    # bR'
                TT(s[2], Ko, K[:, OWN - SW : OWN - SW + OWN_W], alu.is_gt)  # bU'
                TT(s[3], Ko, K[:, OWN + SW : OWN + SW + OWN_W], alu.is_ge)  # bD'
                TT(s[4], s[0], s[1], alu.add)                               # e1
                TT(s[5], s[2], s[3], alu.add)                               # e2
                TT(s[2], s[4], s[5], alu.add)                               # E
                TT(s[3], Ko, ehx[:, 0:OWN_W], alu.is_gt)                    # cUL
                TT(s[4], s[3], s[0], alu.mult)                              # S_ul
                TT(s[5], Ko, ehx[:, 1 : 1 + OWN_W], alu.is_gt)              # cUR
                TT(s[3], s[5], s[1], alu.mult)                              # S_ur
                TT(s[5], Ko, ehx[:, 260 : 260 + OWN_W], alu.is_ge)          # cLL
                TT(s[1], s[5], s[0], alu.mult)                              # S_ll
                TT(s[5], Ko, q[:, :], alu.is_ge)                            # S_lr
                TT(s[0], s[4], s[3], alu.add)                               # S_ul+S_ur
                TT(s[3], s[1], s[5], alu.add)                               # S_ll+S_lr
                TT(s[4], s[0], s[3], alu.add)                               # S
                nc.vector.tensor_scalar(s[5], s[2], -1.0, 1.0, alu.mult, alu.add)  # 1-E
                TT(s[1], s[4], s[5], alu.add)                               # w (raw)
                # zero w at sentinel positions so sign-sourced sums are clean
                nc.vector.tensor_scalar(s[0], Ko, 63.0, 1.0, alu.is_le, alu.mult)
                TT(w[:, :], s[1], s[0], alu.mult)                           # w

            # w2: -w/2 on the ACT-Sign half, w on the DVE-is_le half.
            # Sign half sums give sum(-w/2 * s_t) = chiA(t) - SwA/2, so
            # chi(t) = psum_total(t) - C with C = sum_A(w2) (chi col 63).
            nc.vector.tensor_scalar(w2[:, 0:XSPL], w[:, 0:XSPL], -0.5, None, alu.mult)
            nc.vector.tensor_copy(w2[:, XSPL:OWN_W], w[:, XSPL:OWN_W])

            # ---- threshold loop (mask*w -> PE colsum -> ACT tail) ----
            # sign-sourced decode: sum = Sw - 2*chi(t) with Sw = sum(w) = 1
            with tc.tile_pool(name="thr", bufs=3) as mp:
                pending = []

                def _emit_tail(pt, ppsum):
                    nc.scalar.activation(
                        scr512[:, :], ppsum[:, :], af.Copy, bias=0.0, scale=1.0,
                        accum_out=chi[:, pt : pt + 1],
                    )

                # C = sum_A(w2) per slice -> chi[:, 63] (decode constant)
                cpsum = pp.tile([SPC, 512], dt.float32, tag="ps")
                for c in range((XSPL + 511) // 512):
                    lo = 512 * c
                    hi = min(512 * (c + 1), XSPL)
                    nc.tensor.matmul(
                        cpsum[:, 0 : hi - lo], bselb[:, :], w2[:, lo:hi],
                        start=(c == 0), stop=(hi == XSPL),
                    )
                pending.append((RES - 1, cpsum))

                for t in range(RES - 1):
                    if t in CELL_TS:
                        continue
                    m0a = mp.tile([NPART, XSPL], dt.bfloat16, tag="m0a", bufs=4)
                    m0b = mp.tile([NPART, OWN_W - XSPL], dt.bfloat16, tag="m0b", bufs=4)
                    m1 = mp.tile([NPART, OWN_W], dt.bfloat16, tag="m1", bufs=4)
                    nc.scalar.activation(
                        m0a[:, :], K[:, OWN : OWN + XSPL], af.Sign,
                        bias=biasT[:, t : t + 1], scale=1.0,
                    )
                    nc.vector.tensor_scalar(
                        m0b[:, :], K[:, OWN + XSPL : OWN + OWN_W], float(t),
                        None, alu.is_le,
                    )
                    nc.vector.tensor_tensor(
                        m1[:, 0:XSPL], m0a[:, :], w2[:, 0:XSPL], alu.mult
                    )
                    nc.vector.tensor_tensor(
                        m1[:, XSPL:OWN_W], m0b[:, :], w2[:, XSPL:OWN_W], alu.mult
                    )
                    psum = pp.tile([SPC, 512], dt.float32, tag="ps")
                    for c in range(NMM):
                        lo = 512 * c
                        hi = min(512 * (c + 1), OWN_W)
                        nc.tensor.matmul(
                            psum[:, 0 : hi - lo], bselb[:, :], m1[:, lo:hi],
                            start=(c == 0), stop=(c == NMM - 1),
                        )
                    pending.append((t, psum))
                    if len(pending) >= 3:
                        _emit_tail(*pending.pop(0))
                for pt, ppsum in pending:
                    _emit_tail(pt, ppsum)

            nc.sync.dma_start(chi_dram, chi[:, :])
            nc.sync.dma_start(acts_dram, acts[:, :])

    if legalize:
        _legalize_waits(nc)
    return nc


def _legalize_waits(nc, max_waits: int = 1):
    """This walrus build rejects instructions with more than one sync wait.
    Split excess waits onto preceding same-engine NoOps."""
    import concourse.mybir as mybir

    for f in nc.m.functions:
        for b in f.blocks:
            il = list(b.instructions)
            out, changed = [], False
            for inst in il:
                try:
                    si = inst.sync_info
                except AttributeError:
                    si = None
                waits = list(si.on_wait) if si else []
                if len(waits) > max_waits:
                    head, keep = waits[:-max_waits], waits[-max_waits:]
                    for k, wv in enumerate(head):
                        out.append(
                            mybir.InstNoOp(
                                name=f"{inst.name}-w{k}",
                                engine=inst.engine,
                                sync_info=mybir.SyncInfo(on_wait=[wv], on_update=[]),
                                bass_nofuse=True,
                            )
                        )
                    inst.sync_info = mybir.SyncInfo(
                        on_wait=keep, on_update=list(si.on_update)
                    )
                    changed = True
                out.append(inst)
            if changed:
                b.instructions = out


def make_host_inputs(xcore: np.ndarray):
    """xcore [SPC, H, W] f32 -> packed xi [NPART, WTOT]."""
    xi = np.full((SPC, 2, ROWS, SW), XSENT, dtype=np.float32)
    xh = xcore.reshape(SPC, 2, 64, W)
    xi[:, :, 2:66, 0:W] = xh
    xi[:, 1, 1, 0:W] = xcore[:, 63, :]   # h=1 up-overlap = image row 63
    xi[:, 0, 66, 0:W] = xcore[:, 64, :]  # h=0 down-overlap = image row 64
    return xi.reshape(NPART, WTOT)


def _host_bsel_bias():
    bsel = np.zeros((NPART, 64), dtype=np.float32)
    bsel[np.arange(NPART), np.arange(NPART) // 2] = 1.0
    bias = np.broadcast_to(
        -(np.arange(64, dtype=np.float32) + 0.5), (NPART, 64)
    ).copy()
    return bsel, bias


def _install_ntff_hook():
    import sys, types

    if "antenv.axon_hooks" in sys.modules:
        return
    mod = types.ModuleType("antenv.axon_hooks")
    state = {"hook": None}
    mod.set_axon_ntff_profile_hook = lambda h: state.update(hook=h)
    mod.get_axon_ntff_profile_hook = lambda: state["hook"]
    sys.modules["antenv.axon_hooks"] = mod
    try:
        from trn_agent_boot.trn_boot import _ntff_profile_via_ctypes

        hook = _ntff_profile_via_ctypes("/opt/axon/libaxon_pjrt.so")
        if hook is not None:
            mod.set_axon_ntff_profile_hook(hook)
    except Exception:
        pass


def _run(x: np.ndarray, trace: bool = False):
    from concourse import bass_utils

    if trace:
        _install_ntff_hook()

    x = np.ascontiguousarray(np.asarray(x), dtype=np.float32)
    assert x.shape == (B, C, H, W)

    if "nc" not in _CACHE:
        _CACHE["nc"] = _build_program()
    nc = _CACHE["nc"]

    bsel, bias = _host_bsel_bias()
    flat = x.reshape(SLICES, H, W)
    in_maps = []
    for k in range(NCORES):
        xi = make_host_inputs(flat[k * SPC : (k + 1) * SPC])
        in_maps.append({"xi": xi, "bsel": bsel, "bias": bias})
    res = bass_utils.run_bass_kernel_spmd(
        nc, in_maps, core_ids=list(range(NCORES)), trace=trace
    )

    ecc = np.empty((SLICES, RES), dtype=np.float64)
    for k in range(NCORES):
        chi = res.results[k]["chi"].astype(np.float64)    # [SPC, 64]
        acts = res.results[k]["acts"].astype(np.float64)  # [NPART, 4*ncell]
        a = acts.reshape(SPC, 2, len(CELL_TS), 4).sum(axis=1)
        sl = slice(k * SPC, (k + 1) * SPC)
        for t in range(RES - 1):
            if t in CELL_TS:
                ic = CELL_TS.index(t)
                # counts c = (N - a)/2 per array; widths cancel:
                # chi = (aEh + aEv - aV - aQ)/2
                ecc[sl, t] = (
                    a[:, ic, 1] + a[:, ic, 2] - a[:, ic, 0] - a[:, ic, 3]
                ) / 2.0
            else:
                # split mask: psum total = chi(t) + C, C stored in col 63
                ecc[sl, t] = chi[:, t] - chi[:, 63]
    ecc[:, RES - 1] = 1.0
    return ecc.reshape(B, C, RES).astype(np.float32), res


def kernel(x: np.ndarray) -> np.ndarray:
    out, _ = _run(x, trace=False)
    return out
